# revision 12
# baseline (speedup 1.0000x reference)
"""HSV hue-loss kernel for Trainium2 (Bass/Tile), 8-core data parallel.

Reference computation (per pixel, channels r,g,b in [0,1]):
    hue6 in [0,6):  r-max: (g-b)/d (mod 6);  g-max: 2+(b-r)/d;  b-max: 4+(r-g)/d
    diff6 = |hp6 - ht6|   (hue kept on x6 scale; /6 folded into final scale)
    c6 = diff6            if diff6 < 3
       = diff6 - 3        if diff6 > 3      (== 0 at diff6 == 3)
    loss = sum(c6) / (6 * B*H*W)

Kernel math notes:
  - is_r = (r >= max(g,b)) reproduces maxc==r with the reference's priority.
  - g-vs-b sector (under !is_r) uses mask (g-b) > 0; the g==b sliver falls to
    the b-branch whose value coincides (hue is continuous there).
  - delta = max(|r-g|,|g-b|,|b-r|) == max-min, via two abs_max ops.
  - 1/delta via ACT: rcp = Exp(-Ln(delta + 1e-30)); bias makes delta==0 give
    rcp=9.3e29 and num==0 -> t = 0 (no NaN), matching the reference's h=0.
  - hue is carried as u = h6 - 4 (constant cancels in the difference):
      base u = t + offm, offm = -2*(g-b > 0)  (b-sector: 0, g-sector: -2)
      r-sector overwrite: u = t + 6*(t<0) - 4
  - per-partition accumulation: sum|d| on ACT Abs accum, count(|d|>=3) on a
    DVE tensor_scalar accum; host combines: sum(c6) = sum|d| - 3*count.
"""

import os
import numpy as np

import concourse.bacc as bacc
import concourse.mybir as mybir
import concourse.tile as tile
from concourse.mybir import ActivationFunctionType as AF, AluOpType as OP

BF16 = mybir.dt.bfloat16
F32 = mybir.dt.float32

P = 128  # SBUF partitions


def build_kernel(
    b_local=4, H=512, W=512, F=2048, cast_mode="dma", compute_dt=BF16, reps=1
):
    """Build the single-core Bass program; returns (nc, n_it).

    Each iteration consumes one [P, F] chunk of each of the 6 channel planes
    (predict r,g,b + target r,g,b) and accumulates per-partition partials.
    """
    plane = H * W
    assert plane % (P * F) == 0, (plane, P, F)
    it_per_img = plane // (P * F)
    n_it = b_local * it_per_img
    cdt = compute_dt

    nc = bacc.Bacc("TRN2", target_bir_lowering=False, debug=False)
    pred = nc.dram_tensor("predict", [b_local, 3, H, W], F32, kind="ExternalInput").ap()
    targ = nc.dram_tensor("target", [b_local, 3, H, W], F32, kind="ExternalInput").ap()
    acc_a_out = nc.dram_tensor("acc_a", [P, n_it], F32, kind="ExternalOutput").ap()
    acc_g_out = nc.dram_tensor("acc_g", [P, n_it], F32, kind="ExternalOutput").ap()

    pred_f = pred.rearrange("b c h w -> b c (h w)")
    targ_f = targ.rearrange("b c h w -> b c (h w)")

    from contextlib import ExitStack

    with tile.TileContext(nc) as tc, ExitStack() as ctx:
        in_pool = ctx.enter_context(tc.tile_pool(name="inp", bufs=2))
        work = ctx.enter_context(tc.tile_pool(name="work", bufs=2))
        lnp = ctx.enter_context(tc.tile_pool(name="lnp", bufs=2))
        accp = ctx.enter_context(tc.tile_pool(name="accp", bufs=1))

        acc_a = accp.tile([P, n_it], F32)
        acc_g = accp.tile([P, n_it], F32)
        bias_tiny = accp.tile([P, 1], F32)
        bias_zero = accp.tile([P, 1], F32)
        nc.vector.memset(bias_tiny[:], 1e-30)
        nc.vector.memset(bias_zero[:], 0.0)

        for rep_it in range(reps * n_it):
            it = rep_it % n_it
            b = it // it_per_img
            j = it % it_per_img
            sl = slice(j * P * F, (j + 1) * P * F)

            def load(src, c, tag):
                t = in_pool.tile([P, F], cdt if cast_mode == "dma" else F32, tag=tag)
                ap = src[b, c][sl].rearrange("(p f) -> p f", p=P)
                if cast_mode == "dma":
                    nc.gpsimd.dma_start(t[:], ap)
                else:
                    nc.sync.dma_start(t[:], ap)
                if cast_mode == "act":
                    tb = in_pool.tile([P, F], cdt, tag=tag + "_b")
                    nc.scalar.copy(tb[:], t[:])
                    return tb
                return t

            def hue_u(r, g, b3, sfx):
                """Returns tile holding u = h6 - 4 for one image chunk."""
                h = work.tile([P, F], cdt, tag="h" + sfx)     # rg -> num -> t -> u
                br = work.tile([P, F], cdt, tag="br" + sfx)
                gb = work.tile([P, F], cdt, tag="gb" + sfx)   # -> zos -> zos-4
                gx = work.tile([P, F], cdt, tag="gx" + sfx)   # gbmax -> d1 -> delta
                ir = work.tile([P, F], cdt, tag="ir" + sfx)
                om = work.tile([P, F], cdt, tag="om" + sfx)
                rcp = work.tile([P, F], cdt, tag="rc" + sfx)
                lnd = lnp.tile([P, F], F32, tag="ln" + sfx)

                v = nc.vector
                v.tensor_tensor(h[:], r[:], g[:], OP.subtract)       # rg
                v.tensor_tensor(br[:], b3[:], r[:], OP.subtract)
                v.tensor_tensor(gb[:], g[:], b3[:], OP.subtract)
                v.tensor_tensor(gx[:], g[:], b3[:], OP.max)          # gbmax
                v.tensor_tensor(ir[:], r[:], gx[:], OP.is_ge)        # is_r
                # delta = max(r,g,b) - min(r,g,b); om tile borrowed for min
                v.tensor_tensor(om[:], g[:], b3[:], OP.min)
                v.tensor_tensor(om[:], r[:], om[:], OP.min)          # m
                v.tensor_tensor(gx[:], r[:], gx[:], OP.max)          # M
                v.tensor_tensor(gx[:], gx[:], om[:], OP.subtract)    # delta
                nc.scalar.activation(lnd[:], gx[:], AF.Ln, bias=bias_tiny[:])
                nc.scalar.activation(rcp[:], lnd[:], AF.Exp, bias=bias_zero[:], scale=-1.0)
                # om = 2*(gb>0) in {+0, 2}: mask for the g-sector (bitcast
                # u16 view must not see -0.0) and the *subtractive* offset.
                v.tensor_scalar(om[:], gb[:], 0.0, 2.0, OP.is_gt, OP.mult)
                U16 = mybir.dt.uint16
                v.copy_predicated(h[:], om[:].bitcast(U16), br[:])   # num: g-sector
                v.copy_predicated(h[:], ir[:].bitcast(U16), gb[:])   # num: r-sector
                v.tensor_tensor(h[:], h[:], rcp[:], OP.mult)         # t
                v.tensor_scalar(gb[:], h[:], 0.0, 6.0, OP.is_ge, OP.mult)  # 6*(t>=0)
                v.tensor_scalar_add(gb[:], gb[:], -2.0)              # {-2, 4}
                v.copy_predicated(om[:], ir[:].bitcast(U16), gb[:])  # r-sector offset
                # u = t - om in f32 (reusing the dead lnd tile): the final
                # hue/diff stage carries a bf16->f32 upgrade that cuts the
                # systematic loss bias ~10x (4e-3 -> 4e-4).
                v.tensor_tensor(lnd[:], h[:], om[:], OP.subtract)    # u = t - om
                return lnd

            rp = load(pred_f, 0, "rp")
            gp = load(pred_f, 1, "gp")
            bp = load(pred_f, 2, "bp")
            rt = load(targ_f, 0, "rt")
            gt = load(targ_f, 1, "gt")
            bt = load(targ_f, 2, "bt")

            up = hue_u(rp, gp, bp, "p")
            ut = hue_u(rt, gt, bt, "t")

            nc.vector.tensor_tensor(up[:], up[:], ut[:], OP.subtract)   # d
            nc.scalar.activation(
                ut[:], up[:], AF.Abs, accum_out=acc_a[:, it : it + 1]
            )                                                            # a, sum|d|
            nc.vector.tensor_scalar(
                up[:], ut[:], 3.0, None, OP.is_ge, OP.add,
                accum_out=acc_g[:, it : it + 1],
            )                                                            # count(a>=3)

        nc.sync.dma_start(acc_a_out[:], acc_a[:])
        nc.sync.dma_start(acc_g_out[:], acc_g[:])

    nc.compile()
    return nc, n_it


def hue_u_numpy(r, g, b, dt=np.float32):
    """Golden model of the kernel's per-image math (for sim checks)."""
    r, g, b = (x.astype(dt) for x in (r, g, b))
    rg = (r - g).astype(dt)
    br = (b - r).astype(dt)
    gb = (g - b).astype(dt)
    gx = np.maximum(g, b).astype(dt)
    is_r = (r >= gx).astype(dt)
    m = np.minimum(r, np.minimum(g, b)).astype(dt)
    delta = (np.maximum(r, gx) - m).astype(dt)
    lnd = np.log(delta.astype(np.float32) + 1e-30).astype(np.float32)
    rcp = np.exp(-lnd).astype(dt)
    om = np.where(gb > 0, dt(2.0), dt(0.0))
    num = np.where(om != 0, br, rg)
    num = np.where(is_r != 0, gb, num).astype(dt)
    t = (num * rcp).astype(dt)
    zos = np.where(t >= 0, dt(6.0), dt(0.0))
    zos2 = (zos - dt(2.0)).astype(dt)
    off = np.where(is_r != 0, zos2, om)
    return t.astype(np.float32) - off.astype(np.float32)  # u stage in f32


def loss_numpy(predict, target, dt=np.float32):
    """Golden model of the whole kernel (host-combined)."""
    up = hue_u_numpy(predict[:, 0], predict[:, 1], predict[:, 2], dt)
    ut = hue_u_numpy(target[:, 0], target[:, 1], target[:, 2], dt)
    d = np.abs((up - ut).astype(np.float32))
    n = d.size
    s = d.astype(np.float64).sum()
    cnt = (d >= 3).sum()
    return np.float32((s - 3.0 * cnt) / (6.0 * n))


_CACHE = {}


def kernel(predict: np.ndarray, target: np.ndarray) -> np.ndarray:
    """Full-input entry point: shards batch over 8 cores, returns scalar loss."""
    from concourse.bass_utils import run_bass_kernel_spmd

    B, C, H, W = predict.shape
    n_cores = 8
    bl = B // n_cores
    key = (bl, H, W)
    if key not in _CACHE:
        _CACHE[key] = build_kernel(
            b_local=bl,
            H=H,
            W=W,
            F=int(os.environ.get("HSV_F", "2048")),
            cast_mode=os.environ.get("HSV_CAST", "dma"),
        )
    nc, n_it = _CACHE[key]

    predict = np.ascontiguousarray(predict, dtype=np.float32)
    target = np.ascontiguousarray(target, dtype=np.float32)
    in_maps = [
        {
            "predict": predict[k * bl : (k + 1) * bl],
            "target": target[k * bl : (k + 1) * bl],
        }
        for k in range(n_cores)
    ]
    res = run_bass_kernel_spmd(nc, in_maps, list(range(n_cores))).results

    tot_a = 0.0
    tot_g = 0.0
    for rmap in res:
        tot_a += rmap["acc_a"].astype(np.float64).sum()
        tot_g += rmap["acc_g"].astype(np.float64).sum()
    n = B * H * W
    return np.float32((tot_a - 3.0 * tot_g) / (6.0 * n))


# revision 13
# speedup vs baseline: 146.5447x; 146.5447x over previous
"""HSV hue-loss kernel for Trainium2 (Bass/Tile), 8-core data parallel.

Reference computation (per pixel, channels r,g,b in [0,1]):
    hue6 in [0,6):  r-max: (g-b)/d (mod 6);  g-max: 2+(b-r)/d;  b-max: 4+(r-g)/d
    diff6 = |hp6 - ht6|   (hue kept on x6 scale; /6 folded into final scale)
    c6 = diff6            if diff6 < 3
       = diff6 - 3        if diff6 > 3      (== 0 at diff6 == 3)
    loss = sum(c6) / (6 * B*H*W)

Kernel math notes:
  - is_r = (r >= max(g,b)) reproduces maxc==r with the reference's priority.
  - g-vs-b sector (under !is_r) uses mask (g-b) > 0; the g==b sliver falls to
    the b-branch whose value coincides (hue is continuous there).
  - delta = max(|r-g|,|g-b|,|b-r|) == max-min, via two abs_max ops.
  - 1/delta via ACT: rcp = Exp(-Ln(delta + 1e-30)); bias makes delta==0 give
    rcp=9.3e29 and num==0 -> t = 0 (no NaN), matching the reference's h=0.
  - hue is carried as u = h6 - 4 (constant cancels in the difference):
      base u = t + offm, offm = -2*(g-b > 0)  (b-sector: 0, g-sector: -2)
      r-sector overwrite: u = t + 6*(t<0) - 4
  - per-partition accumulation: sum|d| on ACT Abs accum, count(|d|>=3) on a
    DVE tensor_scalar accum; host combines: sum(c6) = sum|d| - 3*count.
"""

import os
import numpy as np

import concourse.bacc as bacc
import concourse.mybir as mybir
import concourse.tile as tile
from concourse.mybir import ActivationFunctionType as AF, AluOpType as OP

BF16 = mybir.dt.bfloat16
F32 = mybir.dt.float32

P = 128  # SBUF partitions


def build_kernel(
    b_local=4, H=512, W=512, F=2048, cast_mode="dma", compute_dt=BF16, reps=1
):
    """Build the single-core Bass program; returns (nc, n_it).

    Each iteration consumes one [P, F] chunk of each of the 6 channel planes
    (predict r,g,b + target r,g,b) and accumulates per-partition partials.
    """
    plane = H * W
    assert plane % (P * F) == 0, (plane, P, F)
    it_per_img = plane // (P * F)
    n_it = b_local * it_per_img
    cdt = compute_dt

    nc = bacc.Bacc("TRN2", target_bir_lowering=False, debug=False)
    pred = nc.dram_tensor("predict", [b_local, 3, H, W], F32, kind="ExternalInput").ap()
    targ = nc.dram_tensor("target", [b_local, 3, H, W], F32, kind="ExternalInput").ap()
    acc_a_out = nc.dram_tensor("acc_a", [P, n_it], F32, kind="ExternalOutput").ap()
    acc_g_out = nc.dram_tensor("acc_g", [P, n_it], F32, kind="ExternalOutput").ap()

    pred_f = pred.rearrange("b c h w -> b c (h w)")
    targ_f = targ.rearrange("b c h w -> b c (h w)")

    from contextlib import ExitStack

    with tile.TileContext(nc) as tc, ExitStack() as ctx:
        in_pool = ctx.enter_context(tc.tile_pool(name="inp", bufs=2))
        work = ctx.enter_context(tc.tile_pool(name="work", bufs=2))
        lnp = ctx.enter_context(tc.tile_pool(name="lnp", bufs=2))
        accp = ctx.enter_context(tc.tile_pool(name="accp", bufs=1))

        acc_a = accp.tile([P, n_it], F32)
        acc_g = accp.tile([P, n_it], F32)
        bias_tiny = accp.tile([P, 1], F32)
        bias_zero = accp.tile([P, 1], F32)
        nc.vector.memset(bias_tiny[:], 1e-30)
        nc.vector.memset(bias_zero[:], 0.0)

        from contextlib import nullcontext

        # reps>1 repeats the whole compute in a dynamic loop (for timing);
        # the body is rep-independent, so the loop var is unused.
        rep_ctx = tc.For_i(0, reps, 1) if reps > 1 else nullcontext()
        with rep_ctx:
          for it in range(n_it):
            b = it // it_per_img
            j = it % it_per_img
            sl = slice(j * P * F, (j + 1) * P * F)

            def load(src, c, tag):
                t = in_pool.tile([P, F], cdt if cast_mode == "dma" else F32, tag=tag)
                ap = src[b, c][sl].rearrange("(p f) -> p f", p=P)
                if cast_mode == "dma":
                    nc.gpsimd.dma_start(t[:], ap)
                else:
                    nc.sync.dma_start(t[:], ap)
                if cast_mode == "act":
                    tb = in_pool.tile([P, F], cdt, tag=tag + "_b")
                    nc.scalar.copy(tb[:], t[:])
                    return tb
                return t

            def hue_u(r, g, b3, sfx):
                """Returns tile holding u = h6 - 4 for one image chunk."""
                h = work.tile([P, F], cdt, tag="h" + sfx)     # rg -> num -> t -> u
                br = work.tile([P, F], cdt, tag="br" + sfx)
                gb = work.tile([P, F], cdt, tag="gb" + sfx)   # -> zos -> zos-4
                gx = work.tile([P, F], cdt, tag="gx" + sfx)   # gbmax -> d1 -> delta
                ir = work.tile([P, F], cdt, tag="ir" + sfx)
                om = work.tile([P, F], cdt, tag="om" + sfx)
                rcp = work.tile([P, F], cdt, tag="rc" + sfx)
                lnd = lnp.tile([P, F], F32, tag="ln" + sfx)

                v = nc.vector
                v.tensor_tensor(h[:], r[:], g[:], OP.subtract)       # rg
                v.tensor_tensor(br[:], b3[:], r[:], OP.subtract)
                v.tensor_tensor(gb[:], g[:], b3[:], OP.subtract)
                v.tensor_tensor(gx[:], g[:], b3[:], OP.max)          # gbmax
                v.tensor_tensor(ir[:], r[:], gx[:], OP.is_ge)        # is_r
                # delta = max(r,g,b) - min(r,g,b); om tile borrowed for min
                v.tensor_tensor(om[:], g[:], b3[:], OP.min)
                v.tensor_tensor(om[:], r[:], om[:], OP.min)          # m
                v.tensor_tensor(gx[:], r[:], gx[:], OP.max)          # M
                v.tensor_tensor(gx[:], gx[:], om[:], OP.subtract)    # delta
                nc.scalar.activation(lnd[:], gx[:], AF.Ln, bias=bias_tiny[:])
                nc.scalar.activation(rcp[:], lnd[:], AF.Exp, bias=bias_zero[:], scale=-1.0)
                # om = 2*(gb>0) in {+0, 2}: mask for the g-sector (bitcast
                # u16 view must not see -0.0) and the *subtractive* offset.
                v.tensor_scalar(om[:], gb[:], 0.0, 2.0, OP.is_gt, OP.mult)
                U16 = mybir.dt.uint16
                v.copy_predicated(h[:], om[:].bitcast(U16), br[:])   # num: g-sector
                v.copy_predicated(h[:], ir[:].bitcast(U16), gb[:])   # num: r-sector
                v.tensor_tensor(h[:], h[:], rcp[:], OP.mult)         # t
                v.tensor_scalar(gb[:], h[:], 0.0, 6.0, OP.is_ge, OP.mult)  # 6*(t>=0)
                v.tensor_scalar_add(gb[:], gb[:], -2.0)              # {-2, 4}
                v.copy_predicated(om[:], ir[:].bitcast(U16), gb[:])  # r-sector offset
                # u = t - om in f32 (reusing the dead lnd tile): the final
                # hue/diff stage carries a bf16->f32 upgrade that cuts the
                # systematic loss bias ~10x (4e-3 -> 4e-4).
                v.tensor_tensor(lnd[:], h[:], om[:], OP.subtract)    # u = t - om
                return lnd

            rp = load(pred_f, 0, "rp")
            gp = load(pred_f, 1, "gp")
            bp = load(pred_f, 2, "bp")
            rt = load(targ_f, 0, "rt")
            gt = load(targ_f, 1, "gt")
            bt = load(targ_f, 2, "bt")

            up = hue_u(rp, gp, bp, "p")
            ut = hue_u(rt, gt, bt, "t")

            nc.vector.tensor_tensor(up[:], up[:], ut[:], OP.subtract)   # d
            nc.scalar.activation(
                ut[:], up[:], AF.Abs, accum_out=acc_a[:, it : it + 1]
            )                                                            # a, sum|d|
            nc.vector.tensor_scalar(
                up[:], ut[:], 3.0, None, OP.is_ge, OP.add,
                accum_out=acc_g[:, it : it + 1],
            )                                                            # count(a>=3)

        nc.sync.dma_start(acc_a_out[:], acc_a[:])
        nc.sync.dma_start(acc_g_out[:], acc_g[:])

    nc.compile()
    return nc, n_it


def hue_u_numpy(r, g, b, dt=np.float32):
    """Golden model of the kernel's per-image math (for sim checks)."""
    r, g, b = (x.astype(dt) for x in (r, g, b))
    rg = (r - g).astype(dt)
    br = (b - r).astype(dt)
    gb = (g - b).astype(dt)
    gx = np.maximum(g, b).astype(dt)
    is_r = (r >= gx).astype(dt)
    m = np.minimum(r, np.minimum(g, b)).astype(dt)
    delta = (np.maximum(r, gx) - m).astype(dt)
    lnd = np.log(delta.astype(np.float32) + 1e-30).astype(np.float32)
    rcp = np.exp(-lnd).astype(dt)
    om = np.where(gb > 0, dt(2.0), dt(0.0))
    num = np.where(om != 0, br, rg)
    num = np.where(is_r != 0, gb, num).astype(dt)
    t = (num * rcp).astype(dt)
    zos = np.where(t >= 0, dt(6.0), dt(0.0))
    zos2 = (zos - dt(2.0)).astype(dt)
    off = np.where(is_r != 0, zos2, om)
    return t.astype(np.float32) - off.astype(np.float32)  # u stage in f32


def loss_numpy(predict, target, dt=np.float32):
    """Golden model of the whole kernel (host-combined)."""
    up = hue_u_numpy(predict[:, 0], predict[:, 1], predict[:, 2], dt)
    ut = hue_u_numpy(target[:, 0], target[:, 1], target[:, 2], dt)
    d = np.abs((up - ut).astype(np.float32))
    n = d.size
    s = d.astype(np.float64).sum()
    cnt = (d >= 3).sum()
    return np.float32((s - 3.0 * cnt) / (6.0 * n))


_CACHE = {}


def kernel(predict: np.ndarray, target: np.ndarray) -> np.ndarray:
    """Full-input entry point: shards batch over 8 cores, returns scalar loss."""
    from concourse.bass_utils import run_bass_kernel_spmd

    B, C, H, W = predict.shape
    n_cores = 8
    bl = B // n_cores
    key = (bl, H, W)
    if key not in _CACHE:
        _CACHE[key] = build_kernel(
            b_local=bl,
            H=H,
            W=W,
            F=int(os.environ.get("HSV_F", "2048")),
            cast_mode=os.environ.get("HSV_CAST", "dma"),
        )
    nc, n_it = _CACHE[key]

    predict = np.ascontiguousarray(predict, dtype=np.float32)
    target = np.ascontiguousarray(target, dtype=np.float32)
    in_maps = [
        {
            "predict": predict[k * bl : (k + 1) * bl],
            "target": target[k * bl : (k + 1) * bl],
        }
        for k in range(n_cores)
    ]
    res = run_bass_kernel_spmd(nc, in_maps, list(range(n_cores))).results

    tot_a = 0.0
    tot_g = 0.0
    for rmap in res:
        tot_a += rmap["acc_a"].astype(np.float64).sum()
        tot_g += rmap["acc_g"].astype(np.float64).sum()
    n = B * H * W
    return np.float32((tot_a - 3.0 * tot_g) / (6.0 * n))


# revision 17
# speedup vs baseline: 202.3164x; 1.3806x over previous
"""HSV hue-loss kernel for Trainium2 (Bass/Tile), 8-core data parallel.

Reference computation (per pixel, channels r,g,b in [0,1]):
    hue6 in [0,6):  r-max: (g-b)/d (mod 6);  g-max: 2+(b-r)/d;  b-max: 4+(r-g)/d
    diff6 = |hp6 - ht6|   (hue kept on x6 scale; /6 folded into final scale)
    c6 = diff6            if diff6 < 3
       = diff6 - 3        if diff6 > 3      (== 0 at diff6 == 3)
    loss = sum(c6) / (6 * B*H*W)

Kernel math notes:
  - is_r = (r >= max(g,b)) reproduces maxc==r with the reference's priority.
  - g-vs-b sector (under !is_r) uses mask (g-b) > 0; the g==b sliver falls to
    the b-branch whose value coincides (hue is continuous there).
  - delta = max(|r-g|,|g-b|,|b-r|) == max-min, via two abs_max ops.
  - 1/delta via ACT: rcp = Exp(-Ln(delta + 1e-30)); bias makes delta==0 give
    rcp=9.3e29 and num==0 -> t = 0 (no NaN), matching the reference's h=0.
  - hue is carried as u = h6 - 4 (constant cancels in the difference):
      base u = t + offm, offm = -2*(g-b > 0)  (b-sector: 0, g-sector: -2)
      r-sector overwrite: u = t + 6*(t<0) - 4
  - per-partition accumulation: sum|d| on ACT Abs accum, count(|d|>=3) on a
    DVE tensor_scalar accum; host combines: sum(c6) = sum|d| - 3*count.
"""

import os
import numpy as np

import concourse.bacc as bacc
import concourse.mybir as mybir
import concourse.tile as tile
from concourse.mybir import ActivationFunctionType as AF, AluOpType as OP

BF16 = mybir.dt.bfloat16
F32 = mybir.dt.float32

P = 128  # SBUF partitions


def build_kernel(
    b_local=4, H=512, W=512, F=2048, cast_mode="dma", compute_dt=BF16, reps=1
):
    """Build the single-core Bass program; returns (nc, n_it).

    Each iteration consumes one [P, F] chunk of each of the 6 channel planes
    (predict r,g,b + target r,g,b) and accumulates per-partition partials.
    """
    plane = H * W
    assert plane % (P * F) == 0, (plane, P, F)
    it_per_img = plane // (P * F)
    n_it = b_local * it_per_img
    cdt = compute_dt

    nc = bacc.Bacc("TRN2", target_bir_lowering=False, debug=False)
    pred = nc.dram_tensor("predict", [b_local, 3, H, W], F32, kind="ExternalInput").ap()
    targ = nc.dram_tensor("target", [b_local, 3, H, W], F32, kind="ExternalInput").ap()
    acc_a_out = nc.dram_tensor("acc_a", [P, n_it], F32, kind="ExternalOutput").ap()
    acc_g_out = nc.dram_tensor("acc_g", [P, n_it], F32, kind="ExternalOutput").ap()

    pred_f = pred.rearrange("b c h w -> b c (h w)")
    targ_f = targ.rearrange("b c h w -> b c (h w)")

    from contextlib import ExitStack

    with tile.TileContext(nc) as tc, ExitStack() as ctx:
        in_pool = ctx.enter_context(tc.tile_pool(name="inp", bufs=2))
        work = ctx.enter_context(tc.tile_pool(name="work", bufs=2))
        lnp = ctx.enter_context(tc.tile_pool(name="lnp", bufs=2))
        accp = ctx.enter_context(tc.tile_pool(name="accp", bufs=1))

        acc_a = accp.tile([P, n_it], F32)
        acc_g = accp.tile([P, n_it], F32)
        bias_tiny = accp.tile([P, 1], F32)
        bias_zero = accp.tile([P, 1], F32)
        nc.vector.memset(bias_tiny[:], 1e-30)
        nc.vector.memset(bias_zero[:], 0.0)

        from contextlib import nullcontext

        # reps>1 repeats the whole compute in a dynamic loop (for timing);
        # the body is rep-independent, so the loop var is unused.
        rep_ctx = tc.For_i(0, reps, 1) if reps > 1 else nullcontext()
        with rep_ctx:
          for it in range(n_it):
            b = it // it_per_img
            j = it % it_per_img
            sl = slice(j * P * F, (j + 1) * P * F)

            def load(src, c, tag):
                t = in_pool.tile([P, F], cdt if cast_mode == "dma" else F32, tag=tag)
                ap = src[b, c][sl].rearrange("(p f) -> p f", p=P)
                if cast_mode == "dma":
                    nc.gpsimd.dma_start(t[:], ap)
                else:
                    nc.sync.dma_start(t[:], ap)
                if cast_mode == "act":
                    tb = in_pool.tile([P, F], cdt, tag=tag + "_b")
                    nc.scalar.copy(tb[:], t[:])
                    return tb
                return t

            def hue_pre(r, g, b3, sfx):
                """Channel diffs, sector masks, delta for one image chunk."""
                h = work.tile([P, F], cdt, tag="h" + sfx)     # rg -> num -> t
                br = work.tile([P, F], cdt, tag="br" + sfx)
                gb = work.tile([P, F], cdt, tag="gb" + sfx)   # -> zos -> zos-2
                gx = work.tile([P, F], cdt, tag="gx" + sfx)   # gbmax -> M -> delta
                ir = work.tile([P, F], cdt, tag="ir" + sfx)
                om = work.tile([P, F], cdt, tag="om" + sfx)

                v = nc.vector
                v.tensor_tensor(h[:], r[:], g[:], OP.subtract)       # rg
                v.tensor_tensor(br[:], b3[:], r[:], OP.subtract)
                v.tensor_tensor(gb[:], g[:], b3[:], OP.subtract)
                v.tensor_tensor(gx[:], g[:], b3[:], OP.max)          # gbmax
                v.tensor_tensor(ir[:], r[:], gx[:], OP.is_ge)        # is_r
                # delta = max(r,g,b) - min(r,g,b); om tile borrowed for min
                v.tensor_tensor(om[:], g[:], b3[:], OP.min)
                v.tensor_tensor(om[:], r[:], om[:], OP.min)          # m
                v.tensor_tensor(gx[:], r[:], gx[:], OP.max)          # M
                v.tensor_tensor(gx[:], gx[:], om[:], OP.subtract)    # delta
                return h, br, gb, gx, ir, om

            def hue_post(pre, rcp, lnd):
                """Select numerator, apply 1/delta, offset/wrap -> u (f32),
                written into the (dead) lnd tile."""
                h, br, gb, gx, ir, om = pre
                v = nc.vector
                # om = 2*(gb>0) in {+0, 2}: mask for the g-sector (bitcast
                # u16 view must not see -0.0) and the *subtractive* offset.
                v.tensor_scalar(om[:], gb[:], 0.0, 2.0, OP.is_gt, OP.mult)
                U16 = mybir.dt.uint16
                v.copy_predicated(h[:], om[:].bitcast(U16), br[:])   # num: g-sector
                v.copy_predicated(h[:], ir[:].bitcast(U16), gb[:])   # num: r-sector
                v.tensor_tensor(h[:], h[:], rcp[:], OP.mult)         # t
                v.tensor_scalar(gb[:], h[:], 0.0, 6.0, OP.is_ge, OP.mult)  # 6*(t>=0)
                v.tensor_scalar_add(gb[:], gb[:], -2.0)              # {-2, 4}
                v.copy_predicated(om[:], ir[:].bitcast(U16), gb[:])  # r-sector offset
                # u = t - om in f32 (reusing a dead lnd-pool tile): the final
                # hue/diff stage carries a bf16->f32 upgrade that cuts the
                # systematic loss bias ~10x (4e-3 -> 4e-4).
                v.tensor_tensor(lnd[:], h[:], om[:], OP.subtract)    # u = t - om
                return lnd

            rp = load(pred_f, 0, "rp")
            gp = load(pred_f, 1, "gp")
            bp = load(pred_f, 2, "bp")
            rt = load(targ_f, 0, "rt")
            gt = load(targ_f, 1, "gt")
            bt = load(targ_f, 2, "bt")

            pre_p = hue_pre(rp, gp, bp, "p")
            pre_t = hue_pre(rt, gt, bt, "t")
            # ACT work batched by function (avoids activation-table swaps):
            # both Ln, then both Exp, writing rcp in bf16.
            lnd_p = lnp.tile([P, F], F32, tag="lnp2")
            lnd_t = lnp.tile([P, F], F32, tag="lnt2")
            rcp_p = work.tile([P, F], cdt, tag="rcp")
            rcp_t = work.tile([P, F], cdt, tag="rct")
            nc.scalar.activation(lnd_p[:], pre_p[3][:], AF.Ln, bias=bias_tiny[:])
            nc.scalar.activation(lnd_t[:], pre_t[3][:], AF.Ln, bias=bias_tiny[:])
            nc.scalar.activation(
                rcp_p[:], lnd_p[:], AF.Exp, bias=bias_zero[:], scale=-1.0
            )
            nc.scalar.activation(
                rcp_t[:], lnd_t[:], AF.Exp, bias=bias_zero[:], scale=-1.0
            )
            up = hue_post(pre_p, rcp_p, lnd_p)
            ut = hue_post(pre_t, rcp_t, lnd_t)

            nc.vector.tensor_tensor(up[:], up[:], ut[:], OP.subtract)   # d
            nc.scalar.activation(
                ut[:], up[:], AF.Abs, accum_out=acc_a[:, it : it + 1]
            )                                                            # a, sum|d|
            nc.vector.tensor_scalar(
                up[:], ut[:], 3.0, None, OP.is_ge, OP.add,
                accum_out=acc_g[:, it : it + 1],
            )                                                            # count(a>=3)

        nc.sync.dma_start(acc_a_out[:], acc_a[:])
        nc.sync.dma_start(acc_g_out[:], acc_g[:])

    nc.compile()
    return nc, n_it


def hue_u_numpy(r, g, b, dt=np.float32):
    """Golden model of the kernel's per-image math (for sim checks)."""
    r, g, b = (x.astype(dt) for x in (r, g, b))
    rg = (r - g).astype(dt)
    br = (b - r).astype(dt)
    gb = (g - b).astype(dt)
    gx = np.maximum(g, b).astype(dt)
    is_r = (r >= gx).astype(dt)
    m = np.minimum(r, np.minimum(g, b)).astype(dt)
    delta = (np.maximum(r, gx) - m).astype(dt)
    lnd = np.log(delta.astype(np.float32) + 1e-30).astype(np.float32)
    rcp = np.exp(-lnd).astype(dt)
    om = np.where(gb > 0, dt(2.0), dt(0.0))
    num = np.where(om != 0, br, rg)
    num = np.where(is_r != 0, gb, num).astype(dt)
    t = (num * rcp).astype(dt)
    zos = np.where(t >= 0, dt(6.0), dt(0.0))
    zos2 = (zos - dt(2.0)).astype(dt)
    off = np.where(is_r != 0, zos2, om)
    return t.astype(np.float32) - off.astype(np.float32)  # u stage in f32


def loss_numpy(predict, target, dt=np.float32):
    """Golden model of the whole kernel (host-combined)."""
    up = hue_u_numpy(predict[:, 0], predict[:, 1], predict[:, 2], dt)
    ut = hue_u_numpy(target[:, 0], target[:, 1], target[:, 2], dt)
    d = np.abs((up - ut).astype(np.float32))
    n = d.size
    s = d.astype(np.float64).sum()
    cnt = (d >= 3).sum()
    return np.float32((s - 3.0 * cnt) / (6.0 * n))


_CACHE = {}


def kernel(predict: np.ndarray, target: np.ndarray) -> np.ndarray:
    """Full-input entry point: shards batch over 8 cores, returns scalar loss."""
    from concourse.bass_utils import run_bass_kernel_spmd

    B, C, H, W = predict.shape
    n_cores = 8
    bl = B // n_cores
    key = (bl, H, W)
    if key not in _CACHE:
        _CACHE[key] = build_kernel(
            b_local=bl,
            H=H,
            W=W,
            F=int(os.environ.get("HSV_F", "2048")),
            cast_mode=os.environ.get("HSV_CAST", "dma"),
        )
    nc, n_it = _CACHE[key]

    predict = np.ascontiguousarray(predict, dtype=np.float32)
    target = np.ascontiguousarray(target, dtype=np.float32)
    in_maps = [
        {
            "predict": predict[k * bl : (k + 1) * bl],
            "target": target[k * bl : (k + 1) * bl],
        }
        for k in range(n_cores)
    ]
    res = run_bass_kernel_spmd(nc, in_maps, list(range(n_cores))).results

    tot_a = 0.0
    tot_g = 0.0
    for rmap in res:
        tot_a += rmap["acc_a"].astype(np.float64).sum()
        tot_g += rmap["acc_g"].astype(np.float64).sum()
    n = B * H * W
    return np.float32((tot_a - 3.0 * tot_g) / (6.0 * n))
